# revision 56
# baseline (speedup 1.0000x reference)
"""ANI-style element-MLP (MoE routing) kernel for 8 TRN2 NeuronCores.

Strategy:
  - Host: bucket atoms by element (expert). Only ~4/9 of atoms match any
    expert; the rest contribute 0.  Each expert bucket is padded to a fixed
    capacity, split in half, and each half is assigned to one core
    (cores 2e, 2e+1 own expert e).  Per-core inputs are the gathered,
    transposed representation rows [D, S] plus that expert's weights laid
    out in SBUF-ready [128, ...] chunk order.
  - Device: 3-layer MLP as tiled matmuls (features on partitions so biases
    are per-partition ACT bias), softplus on the scalar engine.  The
    softplus -log(2) shift is folded into the next layer's bias on host.
    Output is the per-slot scalar energy [1, S] per core.
  - Host: scatter-add real slots' energies into the per-molecule output [B].

Self-contained: hardcodes problem shapes B=32, N=512, D=384, E=4, H=256.
"""

import ml_dtypes
import numpy as np

import concourse.bass as bass  # noqa: F401  (bass types referenced via bacc/mybir)
import concourse.mybir as mybir
from concourse import bacc
from concourse.bass_utils import run_bass_kernel_spmd
from concourse.hw_specs import get_activation_tables

class _OneActSetBacc(bacc.Bacc):
    """All our ACT functions (Exp, Ln, Identity) live in the
    natural_log_exp_and_others table set, but the stock table-load pass
    assigns each function its first matching set, thrashing ~1.5us table
    loads between sets on every layer.  Force every load to the one set
    that covers all three and drop the now-redundant reloads."""

    _ACT_SET = "natural_log_exp_and_others"

    def insert_act_table_loads(self):
        super().insert_act_table_loads()
        names = list(get_activation_tables(self.m.arch))
        target = names.index(self._ACT_SET)
        for blk in self.main_func.blocks:
            seen_engines = set()
            to_remove = []
            for inst in blk.instructions:
                if isinstance(inst, mybir.InstLoadActFuncSet):
                    if inst.engine in seen_engines and not (inst.has_wait() or inst.has_update()):
                        to_remove.append(inst)
                    else:
                        inst.act_func_set_id = target
                        seen_engines.add(inst.engine)
            for inst in to_remove:
                blk.instructions.remove(inst)

LOG2 = np.float32(np.log(2.0))
B, N, D = 32, 512, 384
E = 4
H1 = H2 = 256
N_CORES = 8
NT = 512  # moving-operand (slot) tile for matmuls; one PSUM bank at f32

F32 = mybir.dt.float32

# Set by test harnesses: PROFILE=True makes kernel() run with NTFF tracing and
# store the profiled NEFF exec time (ns) in LAST_EXEC_NS.
PROFILE = False
TRACE_CORES = [0]
LAST_EXEC_NS = None

_CACHE: dict = {}


BF16 = mybir.dt.bfloat16


def _build(S: int):
    """Raw-Bass per-core graph for S slots (one expert per core).

    Engine plan (explicit semaphores, no Tile):
      sync   : x DMAs in, final out DMA
      scalar : weight/bias DMAs (2nd HWDGE queue), all Exp/Ln activations
      tensor : all matmuls (z1/z2 per slot-chunk + the W3 row, PSUM-aliased)
      vector : +b3 epilogue copy PSUM->SBUF out
    """
    from contextlib import ExitStack

    nc = _OneActSetBacc(None, target_bir_lowering=False)

    x_ext = nc.declare_dram_parameter("x", [128, 3 * S], BF16, isOutput=False)
    wt_ext = nc.declare_dram_parameter("wt", [128, 1282], BF16, isOutput=False)
    bias_ext = nc.declare_dram_parameter("bias", [128, 5], F32, isOutput=False)
    brow_ext = nc.declare_dram_parameter("brow", [1, 512], BF16, isOutput=False)
    out_ext = nc.declare_dram_parameter("out", [1, S], F32, isOutput=True)

    EXP = mybir.ActivationFunctionType.Exp
    LN = mybir.ActivationFunctionType.Ln

    TCH = S // NT  # slot chunks (2 for S=1024)
    if TCH != 2:
        return _build_generic(S)

    with ExitStack() as ctx:
        xt = ctx.enter_context(nc.sbuf_tensor([128, 3 * S], BF16))
        wt = ctx.enter_context(nc.sbuf_tensor([128, 1282], BF16))
        bias = ctx.enter_context(nc.sbuf_tensor([128, 5], F32))
        scratch = ctx.enter_context(nc.sbuf_tensor([1, 16], F32))
        warm = ctx.enter_context(nc.sbuf_tensor([128, NT], BF16))
        ones = ctx.enter_context(nc.sbuf_tensor([1, NT], BF16))
        brow_sb = ctx.enter_context(nc.sbuf_tensor([1, 512], BF16))
        out_sb = ctx.enter_context(nc.sbuf_tensor([1, S], F32))
        t1 = [ctx.enter_context(nc.sbuf_tensor(f"t1_{t}", [128, 2 * NT], F32)) for t in range(TCH)]
        a1 = [ctx.enter_context(nc.sbuf_tensor(f"a1_{t}", [128, 2 * NT], BF16)) for t in range(TCH)]
        t2 = [ctx.enter_context(nc.sbuf_tensor(f"t2_{t}", [128, 2 * NT], F32)) for t in range(TCH)]
        a2 = [ctx.enter_context(nc.sbuf_tensor(f"a2_{t}", [128, 2 * NT], BF16)) for t in range(TCH)]
        z1 = [ctx.enter_context(nc.psum_tensor(f"z1_{t}", [128, 2 * NT], F32)) for t in range(TCH)]
        z2 = [ctx.enter_context(nc.psum_tensor(f"z2_{t}", [128, 2 * NT], F32)) for t in range(TCH)]
        sem_x0 = ctx.enter_context(nc.semaphore("sem_x0"))
        sem_x0b = ctx.enter_context(nc.semaphore("sem_x0b"))
        sem_x1 = ctx.enter_context(nc.semaphore("sem_x1"))
        sem_x1b = ctx.enter_context(nc.semaphore("sem_x1b"))
        sem_w = ctx.enter_context(nc.semaphore("sem_w"))
        sem_b = ctx.enter_context(nc.semaphore("sem_b"))
        sem_o = ctx.enter_context(nc.semaphore("sem_o"))
        sem_mm = ctx.enter_context(nc.semaphore("sem_mm"))
        sem_act = ctx.enter_context(nc.semaphore("sem_act"))
        sem_v = ctx.enter_context(nc.semaphore("sem_v"))
        sem_warm = ctx.enter_context(nc.semaphore("sem_warm"))
        sem_o2 = ctx.enter_context(nc.semaphore("sem_o2"))
        sem_w2 = ctx.enter_context(nc.semaphore("sem_w2"))
        sem_b2 = ctx.enter_context(nc.semaphore("sem_b2"))
        block = ctx.enter_context(nc.Block())

        # the W3 energy row reuses z2[t]'s first bank, partition 0 (its
        # matmuls run only after the Exps have drained z2[t])
        er = [z2[t][0:1, 0:NT] for t in range(TCH)]

        def w1s(d, h):
            return wt[:, (d * 2 + h) * 128 : (d * 2 + h + 1) * 128]

        def w2s(h, k):
            return wt[:, 768 + (h * 2 + k) * 128 : 768 + (h * 2 + k + 1) * 128]

        def w3s(k):
            return wt[:, 1280 + k : 1281 + k]

        @block.sync
        def _(sync):
            # host supplies x pre-laid-out as [128, t*(3*NT) + d*NT + s].
            # x is split across both HWDGE rings (SP + ACT) so the two rings
            # pull in parallel; d0+d1 of each chunk on SP, d2 on ACT.
            sync.dma_start(xt[:, 0 : 2 * NT], x_ext[:, 0 : 2 * NT]).then_inc(sem_x0, 16)
            sync.dma_start(xt[:, 3 * NT : 5 * NT], x_ext[:, 3 * NT : 5 * NT]).then_inc(sem_x1, 16)
            sync.dma_start(bias[:], bias_ext[:]).then_inc(sem_b, 16)
            sync.dma_start(brow_sb[:], brow_ext[:]).then_inc(sem_b2, 16)
            sync.wait_ge(sem_v, 1)
            sync.dma_start(out_ext[:, 0:NT], out_sb[:, 0:NT]).then_inc(sem_o, 16)
            sync.wait_ge(sem_o, 16)

        @block.scalar
        def _(scalar):
            scalar.dma_start(xt[:, 2 * NT : 3 * NT], x_ext[:, 2 * NT : 3 * NT]).then_inc(sem_x0b, 16)
            scalar.dma_start(wt[:, 0:768], wt_ext[:, 0:768]).then_inc(sem_w, 16)
            scalar.dma_start(xt[:, 5 * NT : 6 * NT], x_ext[:, 5 * NT : 6 * NT]).then_inc(sem_x1b, 16)
            scalar.dma_start(wt[:, 768:1282], wt_ext[:, 768:1282]).then_inc(sem_w2, 16)
            # memzero lowers to an ACTIVATE, anchoring the ACT table load
            # before any cross-engine waits
            scalar.memzero(scratch[:])
            # PE sem_mm (coarse, biases included via ones-row matmuls):
            # z1t0=1, z1t1=2, z2t0=3, z2t1=4, er0=5, er1=6.
            # sem_act: li 0..2 -> exp,ln = 2 incs; li 3 -> exp,ln half,ln half.
            for li, zz, tt, aa in (
                (0, z1, t1, a1),
                (1, z1, t1, a1),
                (2, z2, t2, a2),
                (3, z2, t2, a2),
            ):
                t = li % 2
                scalar.wait_ge(sem_mm, li + 1)
                scalar.activation(tt[t][:], zz[t][:], EXP, bias=0.0).then_inc(sem_act, 1)
                scalar.wait_ge(sem_act, 2 * li + 1)  # ACT pipeline RAW: exp fully written
                if li < 3:
                    scalar.activation(aa[t][:], tt[t][:], LN, bias=1.0).then_inc(sem_act, 1)
                else:
                    for k in range(2):
                        scalar.activation(
                            aa[t][:, k * NT : (k + 1) * NT],
                            tt[t][:, k * NT : (k + 1) * NT],
                            LN,
                            bias=1.0,
                        ).then_inc(sem_act, 1)
            # final chunk epilogue on ACT itself: er(1) + b3 -> out, then ship
            ID = mybir.ActivationFunctionType.Identity
            scalar.wait_ge(sem_mm, 6)
            scalar.wait_ge(sem_b, 16)
            scalar.activation(out_sb[:, NT : 2 * NT], er[1], ID, bias=bias[0:1, 4:5]).then_inc(sem_act, 1)
            scalar.wait_ge(sem_act, 10)  # HWDGE reads race the ACT datapath without this
            scalar.dma_start(out_ext[:, NT : 2 * NT], out_sb[:, NT : 2 * NT]).then_inc(sem_o2, 16)
            scalar.wait_ge(sem_o2, 16)

        @block.tensor
        def _(tensor):
            def bias_mms(zz, t, brow_off):
                # z[:, h*NT:+NT] += b_row[h] (x) ones  — K=1 rank-1 update
                for h in range(2):
                    mm = tensor.matmul(
                        zz[t][:, h * NT : (h + 1) * NT],
                        brow_sb[0:1, brow_off + h * 128 : brow_off + (h + 1) * 128],
                        ones[:],
                        start=False,
                        stop=True,
                        skip_group_check=True,
                    )
                mm.then_inc(sem_mm, 1)

            def l1(t, sem_rest):
                # d2 first: it rides the less-loaded ACT ring and lands ~2us
                # before d0/d1; its matmuls overlap the tail of the x DMA
                for h in range(2):
                    tensor.matmul(
                        z1[t][:, h * NT : (h + 1) * NT],
                        w1s(2, h),
                        xt[:, (t * 3 + 2) * NT : (t * 3 + 3) * NT],
                        start=True,
                        stop=False,
                        skip_group_check=True,
                    )
                tensor.wait_ge(sem_rest, 16)
                for h in range(2):
                    for d in range(2):
                        tensor.matmul(
                            z1[t][:, h * NT : (h + 1) * NT],
                            w1s(d, h),
                            xt[:, (t * 3 + d) * NT : (t * 3 + d + 1) * NT],
                            start=False,
                            stop=False,
                            skip_group_check=True,
                        )
                bias_mms(z1, t, 0)

            def l2(t):
                for k in range(2):
                    for h in range(2):
                        tensor.matmul(
                            z2[t][:, k * NT : (k + 1) * NT],
                            w2s(h, k),
                            a1[t][:, h * NT : (h + 1) * NT],
                            start=(h == 0),
                            stop=False,
                            skip_group_check=True,
                        )
                bias_mms(z2, t, 256)

            def l3(t, act_waits):
                for k in range(2):
                    tensor.wait_ge(sem_act, act_waits[k])
                    mm = tensor.matmul(
                        er[t],
                        w3s(k),
                        a2[t][:, k * NT : (k + 1) * NT],
                        start=(k == 0),
                        stop=(k == 1),
                        skip_group_check=True,
                    )
                mm.then_inc(sem_mm, 1)

            # HAM warmup: ~3.5us of dummy matmuls while x streams in
            tensor.wait_ge(sem_warm, 1)
            for _ in range(9):
                tensor.matmul(
                    z1[0][:, 0:NT], warm[:, 0:128], warm[:], start=True, stop=True,
                    skip_group_check=True,
                )
            tensor.wait_ge(sem_w, 16)
            tensor.wait_ge(sem_b2, 16)
            tensor.wait_ge(sem_warm, 2)
            tensor.wait_ge(sem_x0b, 16)
            l1(0, sem_x0)  # -> 1
            tensor.wait_ge(sem_x1b, 16)
            l1(1, sem_x1)  # -> 2
            tensor.wait_ge(sem_w2, 16)
            tensor.wait_ge(sem_act, 2)
            l2(0)  # -> 3
            tensor.wait_ge(sem_act, 4)
            l2(1)  # -> 4
            l3(0, (6, 6))  # -> 5
            l3(1, (8, 9))  # -> 6

        @block.vector
        def _(vector):
            vector.memzero(warm[:]).then_inc(sem_warm, 1)
            vector.wait_ge(sem_warm, 1)
            vector.tensor_scalar_add(ones[:], warm[0:1, 0:NT], 1.0).then_inc(sem_warm, 1)
            vector.wait_ge(sem_b, 16)
            vector.wait_ge(sem_mm, 5)
            vector.tensor_scalar_add(
                out_sb[:, 0:NT], er[0], bias[0:1, 4:5]
            ).then_inc(sem_v, 1)

    nc.finalize()
    return nc


def _build_generic(S: int):
    """Fallback graph for S != 2*NT (pathological element distributions):
    simple sequential per-chunk schedule, PSUM/SBUF reused across chunks."""
    from contextlib import ExitStack

    nc = _OneActSetBacc(None, target_bir_lowering=False)

    x_ext = nc.declare_dram_parameter("x", [128, 3 * S], BF16, isOutput=False)
    wt_ext = nc.declare_dram_parameter("wt", [128, 1282], BF16, isOutput=False)
    bias_ext = nc.declare_dram_parameter("bias", [128, 5], F32, isOutput=False)
    out_ext = nc.declare_dram_parameter("out", [1, S], F32, isOutput=True)

    EXP = mybir.ActivationFunctionType.Exp
    LN = mybir.ActivationFunctionType.Ln
    TCH = S // NT

    with ExitStack() as ctx:
        xt = ctx.enter_context(nc.sbuf_tensor([128, 3 * S], BF16))
        wt = ctx.enter_context(nc.sbuf_tensor([128, 1282], BF16))
        bias = ctx.enter_context(nc.sbuf_tensor([128, 5], F32))
        scratch = ctx.enter_context(nc.sbuf_tensor([1, 16], F32))
        out_sb = ctx.enter_context(nc.sbuf_tensor([1, S], F32))
        t1 = ctx.enter_context(nc.sbuf_tensor([128, 2 * NT], F32))
        a1 = ctx.enter_context(nc.sbuf_tensor([128, 2 * NT], BF16))
        t2 = ctx.enter_context(nc.sbuf_tensor([128, 2 * NT], F32))
        a2 = ctx.enter_context(nc.sbuf_tensor([128, 2 * NT], BF16))
        z1 = ctx.enter_context(nc.psum_tensor([128, 2 * NT], F32))
        z2 = ctx.enter_context(nc.psum_tensor([128, 2 * NT], F32))
        sem_xa = ctx.enter_context(nc.semaphore("sem_xa"))
        sem_xb = ctx.enter_context(nc.semaphore("sem_xb"))
        sem_w = ctx.enter_context(nc.semaphore("sem_w"))
        sem_b = ctx.enter_context(nc.semaphore("sem_b"))
        sem_o = ctx.enter_context(nc.semaphore("sem_o"))
        sem_o2 = ctx.enter_context(nc.semaphore("sem_o2"))
        sem_mm = ctx.enter_context(nc.semaphore("sem_mm"))
        sem_act = ctx.enter_context(nc.semaphore("sem_act"))
        sem_v = ctx.enter_context(nc.semaphore("sem_v"))
        block = ctx.enter_context(nc.Block())

        er = z2[0:1, 0:NT]

        def w1s(d, h):
            return wt[:, (d * 2 + h) * 128 : (d * 2 + h + 1) * 128]

        def w2s(h, k):
            return wt[:, 768 + (h * 2 + k) * 128 : 768 + (h * 2 + k + 1) * 128]

        def w3s(k):
            return wt[:, 1280 + k : 1281 + k]

        n_sync_outs = (TCH + 1) // 2
        n_scalar_outs = TCH // 2

        @block.sync
        def _(sync):
            for t in range(TCH):
                c = 3 * t * NT
                sync.dma_start(xt[:, c : c + 2 * NT], x_ext[:, c : c + 2 * NT]).then_inc(sem_xa, 16)
            sync.dma_start(bias[:], bias_ext[:]).then_inc(sem_b, 16)
            for i, t in enumerate(range(0, TCH, 2)):
                sync.wait_ge(sem_v, t + 1)
                sync.dma_start(out_ext[:, t * NT : (t + 1) * NT], out_sb[:, t * NT : (t + 1) * NT]).then_inc(sem_o, 16)
            sync.wait_ge(sem_o, 16 * n_sync_outs)

        @block.scalar
        def _(scalar):
            scalar.dma_start(wt[:], wt_ext[:]).then_inc(sem_w, 16)
            for t in range(TCH):
                c = (3 * t + 2) * NT
                scalar.dma_start(xt[:, c : c + NT], x_ext[:, c : c + NT]).then_inc(sem_xb, 16)
            scalar.memzero(scratch[:])
            scalar.wait_ge(sem_b, 16)
            # per t: mm incs z1=3t+1, z2=3t+2, er=3t+3; act incs 6 per t
            for t in range(TCH):
                scalar.wait_ge(sem_mm, 3 * t + 1)
                for h in range(2):
                    scalar.activation(
                        t1[:, h * NT : (h + 1) * NT], z1[:, h * NT : (h + 1) * NT],
                        EXP, bias=bias[:, h : h + 1],
                    ).then_inc(sem_act, 1)
                scalar.wait_ge(sem_act, 6 * t + 2)
                scalar.activation(a1[:], t1[:], LN, bias=1.0).then_inc(sem_act, 1)
                scalar.wait_ge(sem_mm, 3 * t + 2)
                for k in range(2):
                    scalar.activation(
                        t2[:, k * NT : (k + 1) * NT], z2[:, k * NT : (k + 1) * NT],
                        EXP, bias=bias[:, 2 + k : 3 + k],
                    ).then_inc(sem_act, 1)
                scalar.wait_ge(sem_act, 6 * t + 5)
                scalar.activation(a2[:], t2[:], LN, bias=1.0).then_inc(sem_act, 1)
            for i, t in enumerate(range(1, TCH, 2)):
                scalar.wait_ge(sem_v, t + 1)
                scalar.dma_start(out_ext[:, t * NT : (t + 1) * NT], out_sb[:, t * NT : (t + 1) * NT]).then_inc(sem_o2, 16)
            if n_scalar_outs:
                scalar.wait_ge(sem_o2, 16 * n_scalar_outs)

        @block.tensor
        def _(tensor):
            tensor.wait_ge(sem_w, 16)
            tensor.wait_ge(sem_xa, 16 * TCH)
            tensor.wait_ge(sem_xb, 16 * TCH)
            for t in range(TCH):
                if t > 0:
                    # z1 reused: exps of chunk t-1 must have drained it
                    tensor.wait_ge(sem_act, 6 * (t - 1) + 2)
                for h in range(2):
                    for d in range(3):
                        mm = tensor.matmul(
                            z1[:, h * NT : (h + 1) * NT], w1s(d, h),
                            xt[:, (t * 3 + d) * NT : (t * 3 + d + 1) * NT],
                            start=(d == 0), stop=(d == 2), skip_group_check=True,
                        )
                mm.then_inc(sem_mm, 1)
                tensor.wait_ge(sem_act, 6 * t + 3)
                if t > 0:
                    # z2 reused: er row of t-1 must be consumed by DVE
                    tensor.wait_ge(sem_v, t)
                for k in range(2):
                    for h in range(2):
                        mm = tensor.matmul(
                            z2[:, k * NT : (k + 1) * NT], w2s(h, k),
                            a1[:, h * NT : (h + 1) * NT],
                            start=(h == 0), stop=(h == 1), skip_group_check=True,
                        )
                mm.then_inc(sem_mm, 1)
                tensor.wait_ge(sem_act, 6 * t + 6)
                for k in range(2):
                    mm = tensor.matmul(
                        er, w3s(k), a2[:, k * NT : (k + 1) * NT],
                        start=(k == 0), stop=(k == 1), skip_group_check=True,
                    )
                mm.then_inc(sem_mm, 1)

        @block.vector
        def _(vector):
            for t in range(TCH):
                vector.wait_ge(sem_mm, 3 * t + 3)
                vector.tensor_scalar_add(
                    out_sb[:, t * NT : (t + 1) * NT], er, bias[0:1, 4:5]
                ).then_inc(sem_v, 1)

    nc.finalize()
    return nc


def kernel(representation, atomic_numbers, elements, W1, b1, W2, b2, W3, b3):
    global LAST_EXEC_NS
    rep = np.asarray(representation, dtype=np.float32)
    an = np.asarray(atomic_numbers).astype(np.int64)
    el = np.asarray(elements).astype(np.int64)
    W1 = np.asarray(W1, dtype=np.float32)
    b1 = np.asarray(b1, dtype=np.float32)
    W2 = np.asarray(W2, dtype=np.float32)
    b2 = np.asarray(b2, dtype=np.float32)
    W3 = np.asarray(W3, dtype=np.float32)
    b3 = np.asarray(b3, dtype=np.float32)

    Bsz, Nn, Dd = rep.shape
    flat = rep.reshape(-1, Dd)
    anf = an.reshape(-1)

    idxs = [np.nonzero(anf == el[e])[0] for e in range(E)]
    counts = [len(ix) for ix in idxs]

    # slots per core; expert capacity = 2*S (two cores per expert)
    S = 1024
    while max(counts) > 2 * S:
        S *= 2

    # fold the shifted-softplus -log(2) into downstream biases
    b2_eff = b2 - LOG2 * W2.sum(axis=1)  # [E, H2]
    b3_eff = b3 - LOG2 * W3.sum(axis=1)  # [E]

    if S not in _CACHE:
        _CACHE[S] = _build(S)
    nc = _CACHE[S]

    in_maps = []
    for c in range(N_CORES):
        e, half = divmod(c, 2)
        ix = idxs[e]
        lo = half * S
        hi = min(len(ix), lo + S)
        bf16 = ml_dtypes.bfloat16
        xs = np.zeros((S, Dd), np.float32)
        if hi > lo:
            xs[: hi - lo] = flat[ix[lo:hi]]
        wt = np.zeros((128, 1282), np.float32)
        wt[:, 0:768] = W1[e].reshape(3, 128, 2, 128).transpose(1, 0, 2, 3).reshape(128, 768)
        wt[:, 768:1280] = W2[e].reshape(2, 128, 2, 128).transpose(1, 0, 2, 3).reshape(128, 512)
        wt[:, 1280:1282] = W3[e].reshape(2, 128).T
        bias = np.zeros((128, 5), np.float32)
        bias[:, 0:2] = b1[e].reshape(2, 128).T
        bias[:, 2:4] = b2_eff[e].reshape(2, 128).T
        bias[0, 4] = b3_eff[e]
        in_maps.append(
            {
                "x": np.ascontiguousarray(
                    xs.T.reshape(3, 128, S // NT, NT).transpose(1, 2, 0, 3).reshape(128, 3 * S)
                ).astype(bf16),
                "wt": wt.astype(bf16),
                "bias": bias,
                "brow": np.concatenate([b1[e], b2_eff[e]]).reshape(1, 512).astype(bf16),
            }
        )

    kwargs = {}
    if PROFILE:
        kwargs = dict(trace=True, trace_cores=list(TRACE_CORES))
    res = run_bass_kernel_spmd(nc, in_maps, core_ids=list(range(N_CORES)), **kwargs)
    LAST_EXEC_NS = res.exec_time_ns

    energies = np.zeros(Bsz, np.float64)
    for c in range(N_CORES):
        e, half = divmod(c, 2)
        ix = idxs[e]
        lo = half * S
        hi = min(len(ix), lo + S)
        if hi <= lo:
            continue
        evals = np.asarray(res.results[c]["out"]).reshape(-1)[: hi - lo]
        np.add.at(energies, ix[lo:hi] // Nn, evals.astype(np.float64))
    return energies.astype(np.float32)


# revision 57
# speedup vs baseline: 1.0836x; 1.0836x over previous
"""ANI-style element-MLP (MoE routing) kernel for 8 TRN2 NeuronCores.

Strategy:
  - Host: bucket atoms by element (expert). Only ~4/9 of atoms match any
    expert; the rest contribute 0.  Each expert bucket is padded to a fixed
    capacity, split in half, and each half is assigned to one core
    (cores 2e, 2e+1 own expert e).  Per-core inputs are the gathered,
    transposed representation rows [D, S] plus that expert's weights laid
    out in SBUF-ready [128, ...] chunk order.
  - Device: 3-layer MLP as tiled matmuls (features on partitions so biases
    are per-partition ACT bias), softplus on the scalar engine.  The
    softplus -log(2) shift is folded into the next layer's bias on host.
    Output is the per-slot scalar energy [1, S] per core.
  - Host: scatter-add real slots' energies into the per-molecule output [B].

Self-contained: hardcodes problem shapes B=32, N=512, D=384, E=4, H=256.
"""

import ml_dtypes
import numpy as np

import concourse.bass as bass  # noqa: F401  (bass types referenced via bacc/mybir)
import concourse.mybir as mybir
from concourse import bacc
from concourse.bass_utils import run_bass_kernel_spmd
from concourse.hw_specs import get_activation_tables

class _OneActSetBacc(bacc.Bacc):
    """All our ACT functions (Exp, Ln, Identity) live in the
    natural_log_exp_and_others table set, but the stock table-load pass
    assigns each function its first matching set, thrashing ~1.5us table
    loads between sets on every layer.  Force every load to the one set
    that covers all three and drop the now-redundant reloads."""

    _ACT_SET = "natural_log_exp_and_others"

    def insert_act_table_loads(self):
        super().insert_act_table_loads()
        names = list(get_activation_tables(self.m.arch))
        target = names.index(self._ACT_SET)
        for blk in self.main_func.blocks:
            seen_engines = set()
            to_remove = []
            for inst in blk.instructions:
                if isinstance(inst, mybir.InstLoadActFuncSet):
                    if inst.engine in seen_engines and not (inst.has_wait() or inst.has_update()):
                        to_remove.append(inst)
                    else:
                        inst.act_func_set_id = target
                        seen_engines.add(inst.engine)
            for inst in to_remove:
                blk.instructions.remove(inst)

LOG2 = np.float32(np.log(2.0))
B, N, D = 32, 512, 384
E = 4
H1 = H2 = 256
N_CORES = 8
NT = 512  # moving-operand (slot) tile for matmuls; one PSUM bank at f32

F32 = mybir.dt.float32

# Set by test harnesses: PROFILE=True makes kernel() run with NTFF tracing and
# store the profiled NEFF exec time (ns) in LAST_EXEC_NS.
PROFILE = False
TRACE_CORES = [0]
LAST_EXEC_NS = None

_CACHE: dict = {}


BF16 = mybir.dt.bfloat16


def _build(S: int):
    """Raw-Bass per-core graph for S slots (one expert per core).

    Engine plan (explicit semaphores, no Tile):
      sync   : x DMAs in, final out DMA
      scalar : weight/bias DMAs (2nd HWDGE queue), all Exp/Ln activations
      tensor : all matmuls (z1/z2 per slot-chunk + the W3 row, PSUM-aliased)
      vector : +b3 epilogue copy PSUM->SBUF out
    """
    from contextlib import ExitStack

    nc = _OneActSetBacc(None, target_bir_lowering=False)

    x_ext = nc.declare_dram_parameter("x", [128, 3 * S], BF16, isOutput=False)
    wt_ext = nc.declare_dram_parameter("wt", [128, 1282], BF16, isOutput=False)
    bias_ext = nc.declare_dram_parameter("bias", [128, 5], F32, isOutput=False)
    brow_ext = nc.declare_dram_parameter("brow", [1, 512], BF16, isOutput=False)
    out_ext = nc.declare_dram_parameter("out", [1, S], F32, isOutput=True)

    EXP = mybir.ActivationFunctionType.Exp
    LN = mybir.ActivationFunctionType.Ln

    TCH = S // NT  # slot chunks (2 for S=1024)
    if TCH != 2:
        return _build_generic(S)

    with ExitStack() as ctx:
        xt = ctx.enter_context(nc.sbuf_tensor([128, 3 * S], BF16))
        wt = ctx.enter_context(nc.sbuf_tensor([128, 1282], BF16))
        bias = ctx.enter_context(nc.sbuf_tensor([128, 5], F32))
        scratch = ctx.enter_context(nc.sbuf_tensor([1, 16], F32))
        warm = ctx.enter_context(nc.sbuf_tensor([128, NT], BF16))
        ones = ctx.enter_context(nc.sbuf_tensor([1, NT], BF16))
        brow_sb = ctx.enter_context(nc.sbuf_tensor([1, 512], BF16))
        out_sb = ctx.enter_context(nc.sbuf_tensor([1, S], F32))
        t1 = [ctx.enter_context(nc.sbuf_tensor(f"t1_{t}", [128, 2 * NT], F32)) for t in range(TCH)]
        a1 = [ctx.enter_context(nc.sbuf_tensor(f"a1_{t}", [128, 2 * NT], BF16)) for t in range(TCH)]
        t2 = [ctx.enter_context(nc.sbuf_tensor(f"t2_{t}", [128, 2 * NT], F32)) for t in range(TCH)]
        a2 = [ctx.enter_context(nc.sbuf_tensor(f"a2_{t}", [128, 2 * NT], BF16)) for t in range(TCH)]
        z1 = [ctx.enter_context(nc.psum_tensor(f"z1_{t}", [128, 2 * NT], F32)) for t in range(TCH)]
        z2 = [ctx.enter_context(nc.psum_tensor(f"z2_{t}", [128, 2 * NT], F32)) for t in range(TCH)]
        sem_x0 = ctx.enter_context(nc.semaphore("sem_x0"))
        sem_x0b = ctx.enter_context(nc.semaphore("sem_x0b"))
        sem_x1 = ctx.enter_context(nc.semaphore("sem_x1"))
        sem_x1b = ctx.enter_context(nc.semaphore("sem_x1b"))
        sem_w = ctx.enter_context(nc.semaphore("sem_w"))
        sem_b = ctx.enter_context(nc.semaphore("sem_b"))
        sem_o = ctx.enter_context(nc.semaphore("sem_o"))
        sem_mm = ctx.enter_context(nc.semaphore("sem_mm"))
        sem_act = ctx.enter_context(nc.semaphore("sem_act"))
        sem_v = ctx.enter_context(nc.semaphore("sem_v"))
        sem_warm = ctx.enter_context(nc.semaphore("sem_warm"))
        sem_o2 = ctx.enter_context(nc.semaphore("sem_o2"))
        sem_w2 = ctx.enter_context(nc.semaphore("sem_w2"))
        sem_b2 = ctx.enter_context(nc.semaphore("sem_b2"))
        block = ctx.enter_context(nc.Block())

        # the W3 energy row reuses z2[t]'s first bank, partition 0 (its
        # matmuls run only after the Exps have drained z2[t])
        er = [z2[t][0:1, 0:NT] for t in range(TCH)]

        def w1s(d, h):
            return wt[:, (d * 2 + h) * 128 : (d * 2 + h + 1) * 128]

        def w2s(h, k):
            return wt[:, 768 + (h * 2 + k) * 128 : 768 + (h * 2 + k + 1) * 128]

        def w3s(k):
            return wt[:, 1280 + k : 1281 + k]

        @block.sync
        def _(sync):
            # host supplies x pre-laid-out as [128, t*(3*NT) + d*NT + s].
            # x is split across both HWDGE rings (SP + ACT) so the two rings
            # pull in parallel; d0+d1 of each chunk on SP, d2 on ACT.
            sync.dma_start(xt[:, 0 : 2 * NT], x_ext[:, 0 : 2 * NT]).then_inc(sem_x0, 16)
            sync.dma_start(xt[:, 3 * NT : 5 * NT], x_ext[:, 3 * NT : 5 * NT]).then_inc(sem_x1, 16)
            sync.dma_start(bias[:], bias_ext[:]).then_inc(sem_b, 16)
            sync.dma_start(brow_sb[:], brow_ext[:]).then_inc(sem_b2, 16)
            sync.wait_ge(sem_v, 1)
            sync.dma_start(out_ext[:, 0:NT], out_sb[:, 0:NT]).then_inc(sem_o, 16)
            sync.wait_ge(sem_o, 16)

        @block.scalar
        def _(scalar):
            scalar.dma_start(xt[:, 2 * NT : 3 * NT], x_ext[:, 2 * NT : 3 * NT]).then_inc(sem_x0b, 16)
            scalar.dma_start(wt[:, 0:768], wt_ext[:, 0:768]).then_inc(sem_w, 16)
            scalar.dma_start(xt[:, 5 * NT : 6 * NT], x_ext[:, 5 * NT : 6 * NT]).then_inc(sem_x1b, 16)
            scalar.dma_start(wt[:, 768:1282], wt_ext[:, 768:1282]).then_inc(sem_w2, 16)
            # memzero lowers to an ACTIVATE, anchoring the ACT table load
            # before any cross-engine waits
            scalar.memzero(scratch[:])
            # PE sem_mm (coarse, biases included via ones-row matmuls):
            # z1t0=1, z1t1=2, z2t0=3, z2t1=4, er0=5, er1=6.
            # sem_act: li 0..2 -> exp,ln = 2 incs; li 3 -> exp,ln half,ln half.
            for li, zz, tt, aa in (
                (0, z1, t1, a1),
                (1, z1, t1, a1),
                (2, z2, t2, a2),
                (3, z2, t2, a2),
            ):
                t = li % 2
                scalar.wait_ge(sem_mm, li + 1)
                scalar.activation(tt[t][:], zz[t][:], EXP, bias=0.0).then_inc(sem_act, 1)
                scalar.wait_ge(sem_act, 2 * li + 1)  # ACT pipeline RAW: exp fully written
                if li < 3:
                    scalar.activation(aa[t][:], tt[t][:], LN, bias=1.0).then_inc(sem_act, 1)
                else:
                    for k in range(2):
                        scalar.activation(
                            aa[t][:, k * NT : (k + 1) * NT],
                            tt[t][:, k * NT : (k + 1) * NT],
                            LN,
                            bias=1.0,
                        ).then_inc(sem_act, 1)
            # final chunk epilogue on ACT itself: er(1) + b3 -> out, then ship
            ID = mybir.ActivationFunctionType.Identity
            scalar.wait_ge(sem_mm, 6)
            scalar.wait_ge(sem_b, 16)
            scalar.activation(out_sb[:, NT : 2 * NT], er[1], ID, bias=bias[0:1, 4:5]).then_inc(sem_act, 1)
            scalar.wait_ge(sem_act, 10)  # HWDGE reads race the ACT datapath without this
            scalar.dma_start(out_ext[:, NT : 2 * NT], out_sb[:, NT : 2 * NT]).then_inc(sem_o2, 16)
            scalar.wait_ge(sem_o2, 16)

        @block.tensor
        def _(tensor):
            def bias_mms(zz, t, brow_off):
                # z[:, h*NT:+NT] += b_row[h] (x) ones  — K=1 rank-1 update
                for h in range(2):
                    mm = tensor.matmul(
                        zz[t][:, h * NT : (h + 1) * NT],
                        brow_sb[0:1, brow_off + h * 128 : brow_off + (h + 1) * 128],
                        ones[:],
                        start=False,
                        stop=True,
                        skip_group_check=True,
                    )
                mm.then_inc(sem_mm, 1)

            def l1(t, sem_rest):
                # d2 first: it rides the less-loaded ACT ring and lands ~2us
                # before d0/d1; its matmuls overlap the tail of the x DMA
                for h in range(2):
                    tensor.matmul(
                        z1[t][:, h * NT : (h + 1) * NT],
                        w1s(2, h),
                        xt[:, (t * 3 + 2) * NT : (t * 3 + 3) * NT],
                        start=True,
                        stop=False,
                        skip_group_check=True,
                    )
                tensor.wait_ge(sem_rest, 16)
                for h in range(2):
                    for d in range(2):
                        tensor.matmul(
                            z1[t][:, h * NT : (h + 1) * NT],
                            w1s(d, h),
                            xt[:, (t * 3 + d) * NT : (t * 3 + d + 1) * NT],
                            start=False,
                            stop=False,
                            skip_group_check=True,
                        )
                bias_mms(z1, t, 0)

            def l2(t):
                for k in range(2):
                    for h in range(2):
                        tensor.matmul(
                            z2[t][:, k * NT : (k + 1) * NT],
                            w2s(h, k),
                            a1[t][:, h * NT : (h + 1) * NT],
                            start=(h == 0),
                            stop=False,
                            skip_group_check=True,
                        )
                bias_mms(z2, t, 256)

            def l3(t, act_waits):
                for k in range(2):
                    tensor.wait_ge(sem_act, act_waits[k])
                    mm = tensor.matmul(
                        er[t],
                        w3s(k),
                        a2[t][:, k * NT : (k + 1) * NT],
                        start=(k == 0),
                        stop=(k == 1),
                        skip_group_check=True,
                    )
                mm.then_inc(sem_mm, 1)

            # HAM warmup: ~3.5us of dummy matmuls while x streams in
            tensor.wait_ge(sem_warm, 1)
            for _ in range(9):
                tensor.matmul(
                    z1[0][:, 0:NT], warm[:, 0:128], warm[:], start=True, stop=True,
                    skip_group_check=True,
                )
            tensor.wait_ge(sem_w, 16)
            tensor.wait_ge(sem_b2, 16)
            tensor.wait_ge(sem_warm, 2)
            tensor.wait_ge(sem_x0b, 16)
            l1(0, sem_x0)  # -> 1
            tensor.wait_ge(sem_x1b, 16)
            l1(1, sem_x1)  # -> 2
            tensor.wait_ge(sem_w2, 16)
            tensor.wait_ge(sem_act, 2)
            l2(0)  # -> 3
            tensor.wait_ge(sem_act, 4)
            l2(1)  # -> 4
            # keep HAM warm through the ln2 waits so the er matmuls run at 2.4GHz
            for _ in range(4):
                tensor.matmul(
                    z1[0][:, 0:NT], warm[:, 0:128], warm[:], start=True, stop=True,
                    skip_group_check=True,
                )
            l3(0, (6, 6))  # -> 5
            for _ in range(2):
                tensor.matmul(
                    z1[0][:, 0:NT], warm[:, 0:128], warm[:], start=True, stop=True,
                    skip_group_check=True,
                )
            l3(1, (8, 9))  # -> 6

        @block.vector
        def _(vector):
            vector.memzero(warm[:]).then_inc(sem_warm, 1)
            vector.wait_ge(sem_warm, 1)
            vector.tensor_scalar_add(ones[:], warm[0:1, 0:NT], 1.0).then_inc(sem_warm, 1)
            vector.wait_ge(sem_b, 16)
            vector.wait_ge(sem_mm, 5)
            vector.tensor_scalar_add(
                out_sb[:, 0:NT], er[0], bias[0:1, 4:5]
            ).then_inc(sem_v, 1)

    nc.finalize()
    return nc


def _build_generic(S: int):
    """Fallback graph for S != 2*NT (pathological element distributions):
    simple sequential per-chunk schedule, PSUM/SBUF reused across chunks."""
    from contextlib import ExitStack

    nc = _OneActSetBacc(None, target_bir_lowering=False)

    x_ext = nc.declare_dram_parameter("x", [128, 3 * S], BF16, isOutput=False)
    wt_ext = nc.declare_dram_parameter("wt", [128, 1282], BF16, isOutput=False)
    bias_ext = nc.declare_dram_parameter("bias", [128, 5], F32, isOutput=False)
    out_ext = nc.declare_dram_parameter("out", [1, S], F32, isOutput=True)

    EXP = mybir.ActivationFunctionType.Exp
    LN = mybir.ActivationFunctionType.Ln
    TCH = S // NT

    with ExitStack() as ctx:
        xt = ctx.enter_context(nc.sbuf_tensor([128, 3 * S], BF16))
        wt = ctx.enter_context(nc.sbuf_tensor([128, 1282], BF16))
        bias = ctx.enter_context(nc.sbuf_tensor([128, 5], F32))
        scratch = ctx.enter_context(nc.sbuf_tensor([1, 16], F32))
        out_sb = ctx.enter_context(nc.sbuf_tensor([1, S], F32))
        t1 = ctx.enter_context(nc.sbuf_tensor([128, 2 * NT], F32))
        a1 = ctx.enter_context(nc.sbuf_tensor([128, 2 * NT], BF16))
        t2 = ctx.enter_context(nc.sbuf_tensor([128, 2 * NT], F32))
        a2 = ctx.enter_context(nc.sbuf_tensor([128, 2 * NT], BF16))
        z1 = ctx.enter_context(nc.psum_tensor([128, 2 * NT], F32))
        z2 = ctx.enter_context(nc.psum_tensor([128, 2 * NT], F32))
        sem_xa = ctx.enter_context(nc.semaphore("sem_xa"))
        sem_xb = ctx.enter_context(nc.semaphore("sem_xb"))
        sem_w = ctx.enter_context(nc.semaphore("sem_w"))
        sem_b = ctx.enter_context(nc.semaphore("sem_b"))
        sem_o = ctx.enter_context(nc.semaphore("sem_o"))
        sem_o2 = ctx.enter_context(nc.semaphore("sem_o2"))
        sem_mm = ctx.enter_context(nc.semaphore("sem_mm"))
        sem_act = ctx.enter_context(nc.semaphore("sem_act"))
        sem_v = ctx.enter_context(nc.semaphore("sem_v"))
        block = ctx.enter_context(nc.Block())

        er = z2[0:1, 0:NT]

        def w1s(d, h):
            return wt[:, (d * 2 + h) * 128 : (d * 2 + h + 1) * 128]

        def w2s(h, k):
            return wt[:, 768 + (h * 2 + k) * 128 : 768 + (h * 2 + k + 1) * 128]

        def w3s(k):
            return wt[:, 1280 + k : 1281 + k]

        n_sync_outs = (TCH + 1) // 2
        n_scalar_outs = TCH // 2

        @block.sync
        def _(sync):
            for t in range(TCH):
                c = 3 * t * NT
                sync.dma_start(xt[:, c : c + 2 * NT], x_ext[:, c : c + 2 * NT]).then_inc(sem_xa, 16)
            sync.dma_start(bias[:], bias_ext[:]).then_inc(sem_b, 16)
            for i, t in enumerate(range(0, TCH, 2)):
                sync.wait_ge(sem_v, t + 1)
                sync.dma_start(out_ext[:, t * NT : (t + 1) * NT], out_sb[:, t * NT : (t + 1) * NT]).then_inc(sem_o, 16)
            sync.wait_ge(sem_o, 16 * n_sync_outs)

        @block.scalar
        def _(scalar):
            scalar.dma_start(wt[:], wt_ext[:]).then_inc(sem_w, 16)
            for t in range(TCH):
                c = (3 * t + 2) * NT
                scalar.dma_start(xt[:, c : c + NT], x_ext[:, c : c + NT]).then_inc(sem_xb, 16)
            scalar.memzero(scratch[:])
            scalar.wait_ge(sem_b, 16)
            # per t: mm incs z1=3t+1, z2=3t+2, er=3t+3; act incs 6 per t
            for t in range(TCH):
                scalar.wait_ge(sem_mm, 3 * t + 1)
                for h in range(2):
                    scalar.activation(
                        t1[:, h * NT : (h + 1) * NT], z1[:, h * NT : (h + 1) * NT],
                        EXP, bias=bias[:, h : h + 1],
                    ).then_inc(sem_act, 1)
                scalar.wait_ge(sem_act, 6 * t + 2)
                scalar.activation(a1[:], t1[:], LN, bias=1.0).then_inc(sem_act, 1)
                scalar.wait_ge(sem_mm, 3 * t + 2)
                for k in range(2):
                    scalar.activation(
                        t2[:, k * NT : (k + 1) * NT], z2[:, k * NT : (k + 1) * NT],
                        EXP, bias=bias[:, 2 + k : 3 + k],
                    ).then_inc(sem_act, 1)
                scalar.wait_ge(sem_act, 6 * t + 5)
                scalar.activation(a2[:], t2[:], LN, bias=1.0).then_inc(sem_act, 1)
            for i, t in enumerate(range(1, TCH, 2)):
                scalar.wait_ge(sem_v, t + 1)
                scalar.dma_start(out_ext[:, t * NT : (t + 1) * NT], out_sb[:, t * NT : (t + 1) * NT]).then_inc(sem_o2, 16)
            if n_scalar_outs:
                scalar.wait_ge(sem_o2, 16 * n_scalar_outs)

        @block.tensor
        def _(tensor):
            tensor.wait_ge(sem_w, 16)
            tensor.wait_ge(sem_xa, 16 * TCH)
            tensor.wait_ge(sem_xb, 16 * TCH)
            for t in range(TCH):
                if t > 0:
                    # z1 reused: exps of chunk t-1 must have drained it
                    tensor.wait_ge(sem_act, 6 * (t - 1) + 2)
                for h in range(2):
                    for d in range(3):
                        mm = tensor.matmul(
                            z1[:, h * NT : (h + 1) * NT], w1s(d, h),
                            xt[:, (t * 3 + d) * NT : (t * 3 + d + 1) * NT],
                            start=(d == 0), stop=(d == 2), skip_group_check=True,
                        )
                mm.then_inc(sem_mm, 1)
                tensor.wait_ge(sem_act, 6 * t + 3)
                if t > 0:
                    # z2 reused: er row of t-1 must be consumed by DVE
                    tensor.wait_ge(sem_v, t)
                for k in range(2):
                    for h in range(2):
                        mm = tensor.matmul(
                            z2[:, k * NT : (k + 1) * NT], w2s(h, k),
                            a1[:, h * NT : (h + 1) * NT],
                            start=(h == 0), stop=(h == 1), skip_group_check=True,
                        )
                mm.then_inc(sem_mm, 1)
                tensor.wait_ge(sem_act, 6 * t + 6)
                for k in range(2):
                    mm = tensor.matmul(
                        er, w3s(k), a2[:, k * NT : (k + 1) * NT],
                        start=(k == 0), stop=(k == 1), skip_group_check=True,
                    )
                mm.then_inc(sem_mm, 1)

        @block.vector
        def _(vector):
            for t in range(TCH):
                vector.wait_ge(sem_mm, 3 * t + 3)
                vector.tensor_scalar_add(
                    out_sb[:, t * NT : (t + 1) * NT], er, bias[0:1, 4:5]
                ).then_inc(sem_v, 1)

    nc.finalize()
    return nc


def kernel(representation, atomic_numbers, elements, W1, b1, W2, b2, W3, b3):
    global LAST_EXEC_NS
    rep = np.asarray(representation, dtype=np.float32)
    an = np.asarray(atomic_numbers).astype(np.int64)
    el = np.asarray(elements).astype(np.int64)
    W1 = np.asarray(W1, dtype=np.float32)
    b1 = np.asarray(b1, dtype=np.float32)
    W2 = np.asarray(W2, dtype=np.float32)
    b2 = np.asarray(b2, dtype=np.float32)
    W3 = np.asarray(W3, dtype=np.float32)
    b3 = np.asarray(b3, dtype=np.float32)

    Bsz, Nn, Dd = rep.shape
    flat = rep.reshape(-1, Dd)
    anf = an.reshape(-1)

    idxs = [np.nonzero(anf == el[e])[0] for e in range(E)]
    counts = [len(ix) for ix in idxs]

    # slots per core; expert capacity = 2*S (two cores per expert)
    S = 1024
    while max(counts) > 2 * S:
        S *= 2

    # fold the shifted-softplus -log(2) into downstream biases
    b2_eff = b2 - LOG2 * W2.sum(axis=1)  # [E, H2]
    b3_eff = b3 - LOG2 * W3.sum(axis=1)  # [E]

    if S not in _CACHE:
        _CACHE[S] = _build(S)
    nc = _CACHE[S]

    in_maps = []
    for c in range(N_CORES):
        e, half = divmod(c, 2)
        ix = idxs[e]
        lo = half * S
        hi = min(len(ix), lo + S)
        bf16 = ml_dtypes.bfloat16
        xs = np.zeros((S, Dd), np.float32)
        if hi > lo:
            xs[: hi - lo] = flat[ix[lo:hi]]
        wt = np.zeros((128, 1282), np.float32)
        wt[:, 0:768] = W1[e].reshape(3, 128, 2, 128).transpose(1, 0, 2, 3).reshape(128, 768)
        wt[:, 768:1280] = W2[e].reshape(2, 128, 2, 128).transpose(1, 0, 2, 3).reshape(128, 512)
        wt[:, 1280:1282] = W3[e].reshape(2, 128).T
        bias = np.zeros((128, 5), np.float32)
        bias[:, 0:2] = b1[e].reshape(2, 128).T
        bias[:, 2:4] = b2_eff[e].reshape(2, 128).T
        bias[0, 4] = b3_eff[e]
        in_maps.append(
            {
                "x": np.ascontiguousarray(
                    xs.T.reshape(3, 128, S // NT, NT).transpose(1, 2, 0, 3).reshape(128, 3 * S)
                ).astype(bf16),
                "wt": wt.astype(bf16),
                "bias": bias,
                "brow": np.concatenate([b1[e], b2_eff[e]]).reshape(1, 512).astype(bf16),
            }
        )

    kwargs = {}
    if PROFILE:
        kwargs = dict(trace=True, trace_cores=list(TRACE_CORES))
    res = run_bass_kernel_spmd(nc, in_maps, core_ids=list(range(N_CORES)), **kwargs)
    LAST_EXEC_NS = res.exec_time_ns

    energies = np.zeros(Bsz, np.float64)
    for c in range(N_CORES):
        e, half = divmod(c, 2)
        ix = idxs[e]
        lo = half * S
        hi = min(len(ix), lo + S)
        if hi <= lo:
            continue
        evals = np.asarray(res.results[c]["out"]).reshape(-1)[: hi - lo]
        np.add.at(energies, ix[lo:hi] // Nn, evals.astype(np.float64))
    return energies.astype(np.float32)


# revision 58
# speedup vs baseline: 1.2288x; 1.1340x over previous
"""ANI-style element-MLP (MoE routing) kernel for 8 TRN2 NeuronCores.

Strategy:
  - Host: bucket atoms by element (expert). Only ~4/9 of atoms match any
    expert; the rest contribute 0.  Each expert bucket is padded to a fixed
    capacity, split in half, and each half is assigned to one core
    (cores 2e, 2e+1 own expert e).  Per-core inputs are the gathered,
    transposed representation rows [D, S] plus that expert's weights laid
    out in SBUF-ready [128, ...] chunk order.
  - Device: 3-layer MLP as tiled matmuls (features on partitions so biases
    are per-partition ACT bias), softplus on the scalar engine.  The
    softplus -log(2) shift is folded into the next layer's bias on host.
    Output is the per-slot scalar energy [1, S] per core.
  - Host: scatter-add real slots' energies into the per-molecule output [B].

Self-contained: hardcodes problem shapes B=32, N=512, D=384, E=4, H=256.
"""

import ml_dtypes
import numpy as np

import concourse.bass as bass  # noqa: F401  (bass types referenced via bacc/mybir)
import concourse.mybir as mybir
from concourse import bacc
from concourse.bass_utils import run_bass_kernel_spmd
from concourse.hw_specs import get_activation_tables

class _OneActSetBacc(bacc.Bacc):
    """All our ACT functions (Exp, Ln, Identity) live in the
    natural_log_exp_and_others table set, but the stock table-load pass
    assigns each function its first matching set, thrashing ~1.5us table
    loads between sets on every layer.  Force every load to the one set
    that covers all three and drop the now-redundant reloads."""

    _ACT_SET = "natural_log_exp_and_others"

    def insert_act_table_loads(self):
        super().insert_act_table_loads()
        names = list(get_activation_tables(self.m.arch))
        target = names.index(self._ACT_SET)
        for blk in self.main_func.blocks:
            seen_engines = set()
            to_remove = []
            for inst in blk.instructions:
                if isinstance(inst, mybir.InstLoadActFuncSet):
                    if inst.engine in seen_engines and not (inst.has_wait() or inst.has_update()):
                        to_remove.append(inst)
                    else:
                        inst.act_func_set_id = target
                        seen_engines.add(inst.engine)
            for inst in to_remove:
                blk.instructions.remove(inst)

LOG2 = np.float32(np.log(2.0))
B, N, D = 32, 512, 384
E = 4
H1 = H2 = 256
N_CORES = 8
NT = 512  # moving-operand (slot) tile for matmuls; one PSUM bank at f32

F32 = mybir.dt.float32

# Set by test harnesses: PROFILE=True makes kernel() run with NTFF tracing and
# store the profiled NEFF exec time (ns) in LAST_EXEC_NS.
PROFILE = False
TRACE_CORES = [0]
LAST_EXEC_NS = None

_CACHE: dict = {}


BF16 = mybir.dt.bfloat16


def _build(S: int):
    """Raw-Bass per-core graph for S slots (one expert per core).

    Engine plan (explicit semaphores, no Tile):
      sync   : x DMAs in, final out DMA
      scalar : weight/bias DMAs (2nd HWDGE queue), all Exp/Ln activations
      tensor : all matmuls (z1/z2 per slot-chunk + the W3 row, PSUM-aliased)
      vector : +b3 epilogue copy PSUM->SBUF out
    """
    from contextlib import ExitStack

    nc = _OneActSetBacc(None, target_bir_lowering=False)

    x_ext = nc.declare_dram_parameter("x", [128, 3 * S], BF16, isOutput=False)
    wt_ext = nc.declare_dram_parameter("wt", [128, 1282], BF16, isOutput=False)
    bias_ext = nc.declare_dram_parameter("bias", [128, 5], F32, isOutput=False)
    brow_ext = nc.declare_dram_parameter("brow", [1, 512], BF16, isOutput=False)
    out_ext = nc.declare_dram_parameter("out", [1, S], F32, isOutput=True)

    EXP = mybir.ActivationFunctionType.Exp
    LN = mybir.ActivationFunctionType.Ln

    TCH = S // NT  # slot chunks (2 for S=1024)
    if TCH != 2:
        return _build_generic(S)

    with ExitStack() as ctx:
        xt = ctx.enter_context(nc.sbuf_tensor([128, 3 * S], BF16))
        wt = ctx.enter_context(nc.sbuf_tensor([128, 1282], BF16))
        bias = ctx.enter_context(nc.sbuf_tensor([128, 5], F32))
        scratch = ctx.enter_context(nc.sbuf_tensor([1, 16], F32))
        warm = ctx.enter_context(nc.sbuf_tensor([128, NT], BF16))
        ones = ctx.enter_context(nc.sbuf_tensor([1, NT], BF16))
        brow_sb = ctx.enter_context(nc.sbuf_tensor([1, 512], BF16))
        out_sb = ctx.enter_context(nc.sbuf_tensor([1, S], F32))
        t1 = [ctx.enter_context(nc.sbuf_tensor(f"t1_{t}", [128, 2 * NT], F32)) for t in range(TCH)]
        a1 = [ctx.enter_context(nc.sbuf_tensor(f"a1_{t}", [128, 2 * NT], BF16)) for t in range(TCH)]
        t2 = [ctx.enter_context(nc.sbuf_tensor(f"t2_{t}", [128, 2 * NT], F32)) for t in range(TCH)]
        a2 = [ctx.enter_context(nc.sbuf_tensor(f"a2_{t}", [128, 2 * NT], BF16)) for t in range(TCH)]
        z1 = [ctx.enter_context(nc.psum_tensor(f"z1_{t}", [128, 2 * NT], F32)) for t in range(TCH)]
        z2 = [ctx.enter_context(nc.psum_tensor(f"z2_{t}", [128, 2 * NT], F32)) for t in range(TCH)]
        sem_x0 = ctx.enter_context(nc.semaphore("sem_x0"))
        sem_x0b = ctx.enter_context(nc.semaphore("sem_x0b"))
        sem_x1 = ctx.enter_context(nc.semaphore("sem_x1"))
        sem_x1b = ctx.enter_context(nc.semaphore("sem_x1b"))
        sem_w = ctx.enter_context(nc.semaphore("sem_w"))
        sem_b = ctx.enter_context(nc.semaphore("sem_b"))
        sem_o = ctx.enter_context(nc.semaphore("sem_o"))
        sem_mm = ctx.enter_context(nc.semaphore("sem_mm"))
        sem_act = ctx.enter_context(nc.semaphore("sem_act"))
        sem_v = ctx.enter_context(nc.semaphore("sem_v"))
        sem_warm = ctx.enter_context(nc.semaphore("sem_warm"))
        sem_o2 = ctx.enter_context(nc.semaphore("sem_o2"))
        sem_w2 = ctx.enter_context(nc.semaphore("sem_w2"))
        sem_b2 = ctx.enter_context(nc.semaphore("sem_b2"))
        block = ctx.enter_context(nc.Block())

        # the W3 energy row reuses z2[t]'s first bank, partition 0 (its
        # matmuls run only after the Exps have drained z2[t])
        er = [z2[t][0:1, 0:NT] for t in range(TCH)]

        def w1s(d, h):
            return wt[:, (d * 2 + h) * 128 : (d * 2 + h + 1) * 128]

        def w2s(h, k):
            return wt[:, 768 + (h * 2 + k) * 128 : 768 + (h * 2 + k + 1) * 128]

        def w3s(k):
            return wt[:, 1280 + k : 1281 + k]

        @block.sync
        def _(sync):
            # host supplies x pre-laid-out as [128, t*(3*NT) + d*NT + s].
            # x is split across both HWDGE rings (SP + ACT) so the two rings
            # pull in parallel; d0+d1 of each chunk on SP, d2 on ACT.
            sync.dma_start(xt[:, 0 : 2 * NT], x_ext[:, 0 : 2 * NT]).then_inc(sem_x0, 16)
            sync.dma_start(xt[:, 3 * NT : 5 * NT], x_ext[:, 3 * NT : 5 * NT]).then_inc(sem_x1, 16)
            sync.dma_start(bias[:], bias_ext[:]).then_inc(sem_b, 16)
            sync.dma_start(brow_sb[:], brow_ext[:]).then_inc(sem_b2, 16)
            sync.wait_ge(sem_v, 1)
            sync.dma_start(out_ext[:, 0:NT], out_sb[:, 0:NT]).then_inc(sem_o, 16)
            sync.wait_ge(sem_o, 16)

        @block.scalar
        def _(scalar):
            scalar.dma_start(xt[:, 2 * NT : 3 * NT], x_ext[:, 2 * NT : 3 * NT]).then_inc(sem_x0b, 16)
            scalar.dma_start(wt[:, 0:768], wt_ext[:, 0:768]).then_inc(sem_w, 16)
            scalar.dma_start(xt[:, 5 * NT : 6 * NT], x_ext[:, 5 * NT : 6 * NT]).then_inc(sem_x1b, 16)
            scalar.dma_start(wt[:, 768:1282], wt_ext[:, 768:1282]).then_inc(sem_w2, 16)
            # memzero lowers to an ACTIVATE, anchoring the ACT table load
            # before any cross-engine waits
            scalar.memzero(scratch[:])
            # PE sem_mm (coarse, biases via ones-row matmuls), h-split t0:
            # z1t0h0=1, z1t0h1=2, z1t1=3, z2t0=4, z2t1=5, er0=6, er1=7.
            # sem_act: e_h0=1,e_h1=2,ln=3 | e=4,ln=5 | e=6,ln=7 | e=8,ln=9,10
            # chunk-0 L1 exp split by h so it starts on half of z1t0
            for h in range(2):
                scalar.wait_ge(sem_mm, h + 1)
                scalar.activation(
                    t1[0][:, h * NT : (h + 1) * NT], z1[0][:, h * NT : (h + 1) * NT], EXP, bias=0.0
                ).then_inc(sem_act, 1)
            scalar.wait_ge(sem_act, 2)  # ACT pipeline RAW
            scalar.activation(a1[0][:], t1[0][:], LN, bias=1.0).then_inc(sem_act, 1)
            for li, zz, tt, aa in (
                (1, z1, t1, a1),
                (2, z2, t2, a2),
                (3, z2, t2, a2),
            ):
                t = li % 2
                scalar.wait_ge(sem_mm, li + 2)
                scalar.activation(tt[t][:], zz[t][:], EXP, bias=0.0).then_inc(sem_act, 1)
                scalar.wait_ge(sem_act, 2 * li + 2)  # ACT pipeline RAW: exp fully written
                if li < 3:
                    scalar.activation(aa[t][:], tt[t][:], LN, bias=1.0).then_inc(sem_act, 1)
                else:
                    for k in range(2):
                        scalar.activation(
                            aa[t][:, k * NT : (k + 1) * NT],
                            tt[t][:, k * NT : (k + 1) * NT],
                            LN,
                            bias=1.0,
                        ).then_inc(sem_act, 1)
            # final chunk epilogue on ACT itself: er(1) + b3 -> out, then ship
            ID = mybir.ActivationFunctionType.Identity
            scalar.wait_ge(sem_mm, 7)
            scalar.wait_ge(sem_b, 16)
            scalar.activation(out_sb[:, NT : 2 * NT], er[1], ID, bias=bias[0:1, 4:5]).then_inc(sem_act, 1)
            scalar.wait_ge(sem_act, 11)  # HWDGE reads race the ACT datapath without this
            scalar.dma_start(out_ext[:, NT : 2 * NT], out_sb[:, NT : 2 * NT]).then_inc(sem_o2, 16)
            scalar.wait_ge(sem_o2, 16)

        @block.tensor
        def _(tensor):
            def bias_mms(zz, t, brow_off):
                # z[:, h*NT:+NT] += b_row[h] (x) ones  — K=1 rank-1 update
                for h in range(2):
                    mm = tensor.matmul(
                        zz[t][:, h * NT : (h + 1) * NT],
                        brow_sb[0:1, brow_off + h * 128 : brow_off + (h + 1) * 128],
                        ones[:],
                        start=False,
                        stop=True,
                        skip_group_check=True,
                    )
                mm.then_inc(sem_mm, 1)

            def l1(t, sem_rest, h_split=False):
                # d2 first: it rides the less-loaded ACT ring and lands ~2us
                # before d0/d1; its matmuls overlap the tail of the x DMA
                for h in range(2):
                    tensor.matmul(
                        z1[t][:, h * NT : (h + 1) * NT],
                        w1s(2, h),
                        xt[:, (t * 3 + 2) * NT : (t * 3 + 3) * NT],
                        start=True,
                        stop=False,
                        skip_group_check=True,
                    )
                tensor.wait_ge(sem_rest, 16)
                for h in range(2):
                    for d in range(2):
                        tensor.matmul(
                            z1[t][:, h * NT : (h + 1) * NT],
                            w1s(d, h),
                            xt[:, (t * 3 + d) * NT : (t * 3 + d + 1) * NT],
                            start=False,
                            stop=False,
                            skip_group_check=True,
                        )
                    if h_split:
                        # finish this h's bias now so the h-half exp can start
                        mm = tensor.matmul(
                            z1[t][:, h * NT : (h + 1) * NT],
                            brow_sb[0:1, h * 128 : (h + 1) * 128],
                            ones[:],
                            start=False,
                            stop=True,
                            skip_group_check=True,
                        )
                        mm.then_inc(sem_mm, 1)
                if not h_split:
                    bias_mms(z1, t, 0)

            def l2(t):
                for k in range(2):
                    for h in range(2):
                        tensor.matmul(
                            z2[t][:, k * NT : (k + 1) * NT],
                            w2s(h, k),
                            a1[t][:, h * NT : (h + 1) * NT],
                            start=(h == 0),
                            stop=False,
                            skip_group_check=True,
                        )
                bias_mms(z2, t, 256)

            def l3(t, act_waits):
                for k in range(2):
                    tensor.wait_ge(sem_act, act_waits[k])
                    mm = tensor.matmul(
                        er[t],
                        w3s(k),
                        a2[t][:, k * NT : (k + 1) * NT],
                        start=(k == 0),
                        stop=(k == 1),
                        skip_group_check=True,
                    )
                mm.then_inc(sem_mm, 1)

            # HAM warmup: ~3.5us of dummy matmuls while x streams in
            tensor.wait_ge(sem_warm, 1)
            for _ in range(9):
                tensor.matmul(
                    z1[0][:, 0:NT], warm[:, 0:128], warm[:], start=True, stop=True,
                    skip_group_check=True,
                )
            tensor.wait_ge(sem_w, 16)
            tensor.wait_ge(sem_b2, 16)
            tensor.wait_ge(sem_warm, 2)
            tensor.wait_ge(sem_x0b, 16)
            l1(0, sem_x0, h_split=True)  # -> 2
            tensor.wait_ge(sem_x1b, 16)
            l1(1, sem_x1)  # -> 3
            tensor.wait_ge(sem_w2, 16)
            tensor.wait_ge(sem_act, 3)
            l2(0)  # -> 4
            tensor.wait_ge(sem_act, 5)
            l2(1)  # -> 5
            # keep HAM warm through the ln2 waits so the er matmuls run at 2.4GHz
            for _ in range(4):
                tensor.matmul(
                    z1[0][:, 0:NT], warm[:, 0:128], warm[:], start=True, stop=True,
                    skip_group_check=True,
                )
            l3(0, (7, 7))  # -> 6
            for _ in range(2):
                tensor.matmul(
                    z1[0][:, 0:NT], warm[:, 0:128], warm[:], start=True, stop=True,
                    skip_group_check=True,
                )
            l3(1, (9, 10))  # -> 7

        @block.vector
        def _(vector):
            vector.memzero(warm[:]).then_inc(sem_warm, 1)
            vector.wait_ge(sem_warm, 1)
            vector.tensor_scalar_add(ones[:], warm[0:1, 0:NT], 1.0).then_inc(sem_warm, 1)
            vector.wait_ge(sem_b, 16)
            vector.wait_ge(sem_mm, 6)
            vector.tensor_scalar_add(
                out_sb[:, 0:NT], er[0], bias[0:1, 4:5]
            ).then_inc(sem_v, 1)

    nc.finalize()
    return nc


def _build_generic(S: int):
    """Fallback graph for S != 2*NT (pathological element distributions):
    simple sequential per-chunk schedule, PSUM/SBUF reused across chunks."""
    from contextlib import ExitStack

    nc = _OneActSetBacc(None, target_bir_lowering=False)

    x_ext = nc.declare_dram_parameter("x", [128, 3 * S], BF16, isOutput=False)
    wt_ext = nc.declare_dram_parameter("wt", [128, 1282], BF16, isOutput=False)
    bias_ext = nc.declare_dram_parameter("bias", [128, 5], F32, isOutput=False)
    out_ext = nc.declare_dram_parameter("out", [1, S], F32, isOutput=True)

    EXP = mybir.ActivationFunctionType.Exp
    LN = mybir.ActivationFunctionType.Ln
    TCH = S // NT

    with ExitStack() as ctx:
        xt = ctx.enter_context(nc.sbuf_tensor([128, 3 * S], BF16))
        wt = ctx.enter_context(nc.sbuf_tensor([128, 1282], BF16))
        bias = ctx.enter_context(nc.sbuf_tensor([128, 5], F32))
        scratch = ctx.enter_context(nc.sbuf_tensor([1, 16], F32))
        out_sb = ctx.enter_context(nc.sbuf_tensor([1, S], F32))
        t1 = ctx.enter_context(nc.sbuf_tensor([128, 2 * NT], F32))
        a1 = ctx.enter_context(nc.sbuf_tensor([128, 2 * NT], BF16))
        t2 = ctx.enter_context(nc.sbuf_tensor([128, 2 * NT], F32))
        a2 = ctx.enter_context(nc.sbuf_tensor([128, 2 * NT], BF16))
        z1 = ctx.enter_context(nc.psum_tensor([128, 2 * NT], F32))
        z2 = ctx.enter_context(nc.psum_tensor([128, 2 * NT], F32))
        sem_xa = ctx.enter_context(nc.semaphore("sem_xa"))
        sem_xb = ctx.enter_context(nc.semaphore("sem_xb"))
        sem_w = ctx.enter_context(nc.semaphore("sem_w"))
        sem_b = ctx.enter_context(nc.semaphore("sem_b"))
        sem_o = ctx.enter_context(nc.semaphore("sem_o"))
        sem_o2 = ctx.enter_context(nc.semaphore("sem_o2"))
        sem_mm = ctx.enter_context(nc.semaphore("sem_mm"))
        sem_act = ctx.enter_context(nc.semaphore("sem_act"))
        sem_v = ctx.enter_context(nc.semaphore("sem_v"))
        block = ctx.enter_context(nc.Block())

        er = z2[0:1, 0:NT]

        def w1s(d, h):
            return wt[:, (d * 2 + h) * 128 : (d * 2 + h + 1) * 128]

        def w2s(h, k):
            return wt[:, 768 + (h * 2 + k) * 128 : 768 + (h * 2 + k + 1) * 128]

        def w3s(k):
            return wt[:, 1280 + k : 1281 + k]

        n_sync_outs = (TCH + 1) // 2
        n_scalar_outs = TCH // 2

        @block.sync
        def _(sync):
            for t in range(TCH):
                c = 3 * t * NT
                sync.dma_start(xt[:, c : c + 2 * NT], x_ext[:, c : c + 2 * NT]).then_inc(sem_xa, 16)
            sync.dma_start(bias[:], bias_ext[:]).then_inc(sem_b, 16)
            for i, t in enumerate(range(0, TCH, 2)):
                sync.wait_ge(sem_v, t + 1)
                sync.dma_start(out_ext[:, t * NT : (t + 1) * NT], out_sb[:, t * NT : (t + 1) * NT]).then_inc(sem_o, 16)
            sync.wait_ge(sem_o, 16 * n_sync_outs)

        @block.scalar
        def _(scalar):
            scalar.dma_start(wt[:], wt_ext[:]).then_inc(sem_w, 16)
            for t in range(TCH):
                c = (3 * t + 2) * NT
                scalar.dma_start(xt[:, c : c + NT], x_ext[:, c : c + NT]).then_inc(sem_xb, 16)
            scalar.memzero(scratch[:])
            scalar.wait_ge(sem_b, 16)
            # per t: mm incs z1=3t+1, z2=3t+2, er=3t+3; act incs 6 per t
            for t in range(TCH):
                scalar.wait_ge(sem_mm, 3 * t + 1)
                for h in range(2):
                    scalar.activation(
                        t1[:, h * NT : (h + 1) * NT], z1[:, h * NT : (h + 1) * NT],
                        EXP, bias=bias[:, h : h + 1],
                    ).then_inc(sem_act, 1)
                scalar.wait_ge(sem_act, 6 * t + 2)
                scalar.activation(a1[:], t1[:], LN, bias=1.0).then_inc(sem_act, 1)
                scalar.wait_ge(sem_mm, 3 * t + 2)
                for k in range(2):
                    scalar.activation(
                        t2[:, k * NT : (k + 1) * NT], z2[:, k * NT : (k + 1) * NT],
                        EXP, bias=bias[:, 2 + k : 3 + k],
                    ).then_inc(sem_act, 1)
                scalar.wait_ge(sem_act, 6 * t + 5)
                scalar.activation(a2[:], t2[:], LN, bias=1.0).then_inc(sem_act, 1)
            for i, t in enumerate(range(1, TCH, 2)):
                scalar.wait_ge(sem_v, t + 1)
                scalar.dma_start(out_ext[:, t * NT : (t + 1) * NT], out_sb[:, t * NT : (t + 1) * NT]).then_inc(sem_o2, 16)
            if n_scalar_outs:
                scalar.wait_ge(sem_o2, 16 * n_scalar_outs)

        @block.tensor
        def _(tensor):
            tensor.wait_ge(sem_w, 16)
            tensor.wait_ge(sem_xa, 16 * TCH)
            tensor.wait_ge(sem_xb, 16 * TCH)
            for t in range(TCH):
                if t > 0:
                    # z1 reused: exps of chunk t-1 must have drained it
                    tensor.wait_ge(sem_act, 6 * (t - 1) + 2)
                for h in range(2):
                    for d in range(3):
                        mm = tensor.matmul(
                            z1[:, h * NT : (h + 1) * NT], w1s(d, h),
                            xt[:, (t * 3 + d) * NT : (t * 3 + d + 1) * NT],
                            start=(d == 0), stop=(d == 2), skip_group_check=True,
                        )
                mm.then_inc(sem_mm, 1)
                tensor.wait_ge(sem_act, 6 * t + 3)
                if t > 0:
                    # z2 reused: er row of t-1 must be consumed by DVE
                    tensor.wait_ge(sem_v, t)
                for k in range(2):
                    for h in range(2):
                        mm = tensor.matmul(
                            z2[:, k * NT : (k + 1) * NT], w2s(h, k),
                            a1[:, h * NT : (h + 1) * NT],
                            start=(h == 0), stop=(h == 1), skip_group_check=True,
                        )
                mm.then_inc(sem_mm, 1)
                tensor.wait_ge(sem_act, 6 * t + 6)
                for k in range(2):
                    mm = tensor.matmul(
                        er, w3s(k), a2[:, k * NT : (k + 1) * NT],
                        start=(k == 0), stop=(k == 1), skip_group_check=True,
                    )
                mm.then_inc(sem_mm, 1)

        @block.vector
        def _(vector):
            for t in range(TCH):
                vector.wait_ge(sem_mm, 3 * t + 3)
                vector.tensor_scalar_add(
                    out_sb[:, t * NT : (t + 1) * NT], er, bias[0:1, 4:5]
                ).then_inc(sem_v, 1)

    nc.finalize()
    return nc


def kernel(representation, atomic_numbers, elements, W1, b1, W2, b2, W3, b3):
    global LAST_EXEC_NS
    rep = np.asarray(representation, dtype=np.float32)
    an = np.asarray(atomic_numbers).astype(np.int64)
    el = np.asarray(elements).astype(np.int64)
    W1 = np.asarray(W1, dtype=np.float32)
    b1 = np.asarray(b1, dtype=np.float32)
    W2 = np.asarray(W2, dtype=np.float32)
    b2 = np.asarray(b2, dtype=np.float32)
    W3 = np.asarray(W3, dtype=np.float32)
    b3 = np.asarray(b3, dtype=np.float32)

    Bsz, Nn, Dd = rep.shape
    flat = rep.reshape(-1, Dd)
    anf = an.reshape(-1)

    idxs = [np.nonzero(anf == el[e])[0] for e in range(E)]
    counts = [len(ix) for ix in idxs]

    # slots per core; expert capacity = 2*S (two cores per expert)
    S = 1024
    while max(counts) > 2 * S:
        S *= 2

    # fold the shifted-softplus -log(2) into downstream biases
    b2_eff = b2 - LOG2 * W2.sum(axis=1)  # [E, H2]
    b3_eff = b3 - LOG2 * W3.sum(axis=1)  # [E]

    if S not in _CACHE:
        _CACHE[S] = _build(S)
    nc = _CACHE[S]

    in_maps = []
    for c in range(N_CORES):
        e, half = divmod(c, 2)
        ix = idxs[e]
        lo = half * S
        hi = min(len(ix), lo + S)
        bf16 = ml_dtypes.bfloat16
        xs = np.zeros((S, Dd), np.float32)
        if hi > lo:
            xs[: hi - lo] = flat[ix[lo:hi]]
        wt = np.zeros((128, 1282), np.float32)
        wt[:, 0:768] = W1[e].reshape(3, 128, 2, 128).transpose(1, 0, 2, 3).reshape(128, 768)
        wt[:, 768:1280] = W2[e].reshape(2, 128, 2, 128).transpose(1, 0, 2, 3).reshape(128, 512)
        wt[:, 1280:1282] = W3[e].reshape(2, 128).T
        bias = np.zeros((128, 5), np.float32)
        bias[:, 0:2] = b1[e].reshape(2, 128).T
        bias[:, 2:4] = b2_eff[e].reshape(2, 128).T
        bias[0, 4] = b3_eff[e]
        in_maps.append(
            {
                "x": np.ascontiguousarray(
                    xs.T.reshape(3, 128, S // NT, NT).transpose(1, 2, 0, 3).reshape(128, 3 * S)
                ).astype(bf16),
                "wt": wt.astype(bf16),
                "bias": bias,
                "brow": np.concatenate([b1[e], b2_eff[e]]).reshape(1, 512).astype(bf16),
            }
        )

    kwargs = {}
    if PROFILE:
        kwargs = dict(trace=True, trace_cores=list(TRACE_CORES))
    res = run_bass_kernel_spmd(nc, in_maps, core_ids=list(range(N_CORES)), **kwargs)
    LAST_EXEC_NS = res.exec_time_ns

    energies = np.zeros(Bsz, np.float64)
    for c in range(N_CORES):
        e, half = divmod(c, 2)
        ix = idxs[e]
        lo = half * S
        hi = min(len(ix), lo + S)
        if hi <= lo:
            continue
        evals = np.asarray(res.results[c]["out"]).reshape(-1)[: hi - lo]
        np.add.at(energies, ix[lo:hi] // Nn, evals.astype(np.float64))
    return energies.astype(np.float32)
